# revision 4
# baseline (speedup 1.0000x reference)
"""AttentionBlock kernel for 8 Trainium2 NeuronCores.

Reference computation (per batch b):
    h = GroupNorm32(x);  q,k,v = 1x1 conv(h);  single-head attention over
    hw=4096 tokens with C=512 channels;  out = x + proj(attn_out).

Sharding: 8 cores = 4 batches x 2 query-halves. Each core gets its batch's
x pre-rotated so its 2048 query tokens sit at columns [0, 2048) (attention
and groupnorm are permutation-invariant over tokens, so rotating keys/values
together is exact). Each core computes groupnorm + K/V for all 4096 tokens
and Q/attention/proj for its 2048 queries.

All matmuls run as float32r (full-rate fp32 mode, ~1e-4 rounding).
"""
import sys

for _p in ("/opt/trn_rl_repo", "/root/.axon_site/_ro/trn_rl_repo"):
    if _p not in sys.path:
        sys.path.append(_p)

import numpy as np

import concourse.bass as bass  # noqa: F401  (registers types)
import concourse.tile as tile
from concourse import bacc, mybir
from contextlib import ExitStack

F32 = mybir.dt.float32
F32R = mybir.dt.float32r

B, C, Hh, Ww = 4, 512, 64, 64
T = Hh * Ww            # 4096 tokens
HALF = T // 2          # 2048 queries per core
CT = C // 128          # 4 channel tiles
NCHUNK = T // 512      # 8 column chunks
NQCHUNK = HALF // 512  # 4 query chunks
NITILE = HALF // 128   # 16 query i-tiles
NJT = T // 128         # 32 key j-tiles
NG_LOCAL = 8           # groups per 128-channel tile (group size 16)
EPS = 1e-5

_CACHE = {}


def _emit(nc):
    x_l = nc.declare_dram_parameter("x_local", [C, T], F32, isOutput=False)
    wqT = nc.declare_dram_parameter("wqT", [C, C], F32R, isOutput=False)
    wkT = nc.declare_dram_parameter("wkT", [C, C], F32R, isOutput=False)
    wvT = nc.declare_dram_parameter("wvT", [C, C], F32R, isOutput=False)
    wpT = nc.declare_dram_parameter("wpT", [C, C], F32R, isOutput=False)
    qb_d = nc.declare_dram_parameter("qb_cols", [CT, 128, 1], F32, isOutput=False)
    kb_d = nc.declare_dram_parameter("kb_cols", [CT, 128, 1], F32, isOutput=False)
    vb_d = nc.declare_dram_parameter("vb_row", [1, C], F32R, isOutput=False)
    pb_d = nc.declare_dram_parameter("pb_row", [1, C], F32R, isOutput=False)
    gam_d = nc.declare_dram_parameter("gam_cols", [CT, 128, 1], F32, isOutput=False)
    bet_d = nc.declare_dram_parameter("bet_cols", [CT, 128, 1], F32, isOutput=False)
    m16_d = nc.declare_dram_parameter("mask16", [128, NG_LOCAL], F32, isOutput=False)
    mbc_d = nc.declare_dram_parameter("maskbc", [NG_LOCAL, 128], F32, isOutput=False)
    id_d = nc.declare_dram_parameter("ident", [128, 128], F32R, isOutput=False)
    onc_d = nc.declare_dram_parameter("ones_col", [128, 2], F32R, isOutput=False)
    onr_d = nc.declare_dram_parameter("ones_row", [1, 512], F32R, isOutput=False)
    out_l = nc.declare_dram_parameter("out_local", [C, HALF], F32, isOutput=True)

    q_dram = nc.dram_tensor("q_scratch", [C, HALF], F32R)

    Exp = mybir.ActivationFunctionType.Exp
    Sqrt = mybir.ActivationFunctionType.Sqrt
    Alu = mybir.AluOpType

    with tile.TileContext(nc) as tc, ExitStack() as ctx:
        consts = ctx.enter_context(tc.tile_pool(name="consts", bufs=1))
        wp_pool = ctx.enter_context(tc.tile_pool(name="wp", bufs=CT))
        k_pool = ctx.enter_context(tc.tile_pool(name="K", bufs=CT))
        v_pool = ctx.enter_context(tc.tile_pool(name="V", bufs=NJT))

        # ---- constants -------------------------------------------------
        m16 = consts.tile([128, NG_LOCAL], F32, tag="m16")
        nc.sync.dma_start(out=m16, in_=m16_d[:, :])
        mbc = consts.tile([NG_LOCAL, 128], F32, tag="mbc")
        nc.sync.dma_start(out=mbc, in_=mbc_d[:, :])
        ident = consts.tile([128, 128], F32R, tag="ident")
        nc.sync.dma_start(out=ident, in_=id_d[:, :])
        ones_c = consts.tile([128, 2], F32R, tag="ones_c")
        nc.sync.dma_start(out=ones_c, in_=onc_d[:, :])
        ones_r = consts.tile([1, 512], F32R, tag="ones_r")
        nc.sync.dma_start(out=ones_r, in_=onr_d[:, :])
        vb = consts.tile([1, C], F32R, tag="vb")
        nc.sync.dma_start(out=vb, in_=vb_d[:, :])
        pb = consts.tile([1, C], F32R, tag="pb")
        nc.sync.dma_start(out=pb, in_=pb_d[:, :])
        gam = consts.tile([128, CT], F32, tag="gam")
        bet = consts.tile([128, CT], F32, tag="bet")
        qb = consts.tile([128, CT], F32, tag="qb")
        kb = consts.tile([128, CT], F32, tag="kb")
        for t in range(CT):
            nc.sync.dma_start(out=gam[:, t:t + 1], in_=gam_d[t])
            nc.sync.dma_start(out=bet[:, t:t + 1], in_=bet_d[t])
            nc.sync.dma_start(out=qb[:, t:t + 1], in_=qb_d[t])
            nc.sync.dma_start(out=kb[:, t:t + 1], in_=kb_d[t])
        eps8 = consts.tile([NG_LOCAL, 1], F32, tag="eps8")
        nc.vector.memset(eps8, EPS)
        # groupnorm per-channel affine (filled by phase A)
        Ac = consts.tile([128, CT], F32, tag="Ac")
        Bc = consts.tile([128, CT], F32, tag="Bc")

        # ---- phase A: groupnorm statistics -----------------------------
        with tc.tile_pool(name="phA", bufs=6) as pha, \
             tc.tile_pool(name="phA_st", bufs=CT) as pst, \
             tc.tile_pool(name="phA_sm", bufs=2) as psm, \
             tc.tile_pool(name="phA_ps", bufs=1, space="PSUM") as pps:
            stats = [pst.tile([128, NCHUNK, 6], F32, tag="st", name="st") for _ in range(CT)]
            for jc in range(NCHUNK):
                for ci in range(CT):
                    xt = pha.tile([128, 512], F32, tag="xa")
                    nc.sync.dma_start(
                        out=xt, in_=x_l[128 * ci:128 * (ci + 1), 512 * jc:512 * (jc + 1)])
                    nc.vector.bn_stats(out=stats[ci][:, jc, :], in_=xt)
            # per-channel mean / E[x^2] -> per-group stats via PE mask matmuls
            ps_gm = pps.tile([NG_LOCAL, CT], F32, tag="gm")
            ps_gq = pps.tile([NG_LOCAL, CT], F32, tag="gq")
            for ci in range(CT):
                mv = psm.tile([128, 2], F32, tag="mv")
                nc.vector.bn_aggr(out=mv, in_=stats[ci])
                msq = psm.tile([128, 1], F32, tag="msq")
                nc.vector.tensor_mul(msq, mv[:, 0:1], mv[:, 0:1])
                qp = psm.tile([128, 1], F32, tag="qp")
                nc.vector.tensor_add(qp, mv[:, 1:2], msq)
                nc.tensor.matmul(ps_gm[:, ci:ci + 1], m16, mv[:, 0:1],
                                 start=(ci == 0), stop=(ci == CT - 1))
                nc.tensor.matmul(ps_gq[:, ci:ci + 1], m16, qp,
                                 start=(ci == 0), stop=(ci == CT - 1))
            sgm = psm.tile([NG_LOCAL, CT], F32, tag="sgm")
            nc.vector.tensor_copy(sgm, ps_gm)
            gvar = psm.tile([NG_LOCAL, CT], F32, tag="gvar")
            nc.vector.tensor_mul(gvar, sgm, sgm)
            nc.vector.tensor_sub(gvar, ps_gq, gvar)
            grstd = psm.tile([NG_LOCAL, CT], F32, tag="grstd")
            nc.scalar.activation(out=grstd, in_=gvar, func=Sqrt, bias=eps8, scale=1.0)
            nc.vector.reciprocal(grstd, grstd)
            # broadcast group stats back to channels, fold gamma/beta
            for ci in range(CT):
                ps_bm = pps.tile([128, 1], F32, tag="bm")
                ps_br = pps.tile([128, 1], F32, tag="br")
                nc.tensor.matmul(ps_bm, mbc, sgm[:, ci:ci + 1], start=True, stop=True)
                nc.tensor.matmul(ps_br, mbc, grstd[:, ci:ci + 1], start=True, stop=True)
                nc.vector.tensor_mul(Ac[:, ci:ci + 1], ps_br, gam[:, ci:ci + 1])
                tmp = psm.tile([128, 1], F32, tag="tmp")
                nc.vector.tensor_mul(tmp, ps_bm, Ac[:, ci:ci + 1])
                nc.vector.tensor_sub(Bc[:, ci:ci + 1], bet[:, ci:ci + 1], tmp)

        # ---- phase B: h = affine(x); K, V^T, Q projections -------------
        K_sb = [k_pool.tile([128, T], F32R, tag="K", name="K") for _ in range(CT)]
        V_sb = [v_pool.tile([128, 512], F32R, tag="V", name="V") for _ in range(NJT)]
        wp_sb = [wp_pool.tile([128, C], F32R, tag="wpT", name="wpT") for _ in range(CT)]
        for ci in range(CT):
            nc.sync.dma_start(out=wp_sb[ci], in_=wpT[128 * ci:128 * (ci + 1), :])

        with tc.tile_pool(name="phB_w", bufs=3 * CT) as pbw, \
             tc.tile_pool(name="phB_x", bufs=6) as pbx, \
             tc.tile_pool(name="phB_h", bufs=6) as pbh, \
             tc.tile_pool(name="phB_q", bufs=2) as pbq, \
             tc.tile_pool(name="phB_ps", bufs=3, space="PSUM") as pbp:
            wq_sb = [pbw.tile([128, C], F32R, tag="wT", name="wT") for _ in range(CT)]
            wk_sb = [pbw.tile([128, C], F32R, tag="wT", name="wT") for _ in range(CT)]
            wv_sb = [pbw.tile([128, C], F32R, tag="wT", name="wT") for _ in range(CT)]
            for ci in range(CT):
                nc.sync.dma_start(out=wq_sb[ci], in_=wqT[128 * ci:128 * (ci + 1), :])
                nc.sync.dma_start(out=wk_sb[ci], in_=wkT[128 * ci:128 * (ci + 1), :])
                nc.sync.dma_start(out=wv_sb[ci], in_=wvT[128 * ci:128 * (ci + 1), :])

            for jc in range(NCHUNK):
                cs = slice(512 * jc, 512 * (jc + 1))
                hj = []
                for ci in range(CT):
                    xt = pbx.tile([128, 512], F32, tag="xb")
                    nc.sync.dma_start(out=xt, in_=x_l[128 * ci:128 * (ci + 1), cs])
                    ht = pbh.tile([128, 512], F32R, tag="hb")
                    nc.vector.tensor_scalar(
                        out=ht, in0=xt, scalar1=Ac[:, ci:ci + 1],
                        scalar2=Bc[:, ci:ci + 1], op0=Alu.mult, op1=Alu.add)
                    hj.append(ht)
                # K[:, chunk]
                for co in range(CT):
                    ps = pbp.tile([128, 512], F32, tag="psb")
                    for ci in range(CT):
                        nc.tensor.matmul(
                            ps, wk_sb[ci][:, 128 * co:128 * (co + 1)], hj[ci],
                            start=(ci == 0), stop=(ci == CT - 1))
                    nc.vector.tensor_scalar(
                        out=K_sb[co][:, cs], in0=ps, scalar1=kb[:, co:co + 1],
                        scalar2=None, op0=Alu.add)
                # V^T tiles (4 per chunk)
                for ti in range(4):
                    jt = 4 * jc + ti
                    ps = pbp.tile([128, 512], F32, tag="psb")
                    for ci in range(CT):
                        nc.tensor.matmul(
                            ps, hj[ci][:, 128 * ti:128 * (ti + 1)], wv_sb[ci],
                            start=(ci == 0), stop=False)
                    nc.tensor.matmul(ps, ones_r[0:1, 0:128], vb[0:1, :],
                                     start=False, stop=True)
                    nc.vector.tensor_copy(V_sb[jt], ps)
                # Q[:, chunk] (first half only) -> DRAM scratch
                if jc < NQCHUNK:
                    for co in range(CT):
                        ps = pbp.tile([128, 512], F32, tag="psb")
                        for ci in range(CT):
                            nc.tensor.matmul(
                                ps, wq_sb[ci][:, 128 * co:128 * (co + 1)], hj[ci],
                                start=(ci == 0), stop=(ci == CT - 1))
                        qt = pbq.tile([128, 512], F32R, tag="qs")
                        nc.vector.tensor_scalar(
                            out=qt, in0=ps, scalar1=qb[:, co:co + 1],
                            scalar2=None, op0=Alu.add)
                        nc.sync.dma_start(
                            out=q_dram[128 * co:128 * (co + 1), cs], in_=qt)

        # ---- phase C: attention + proj + residual ----------------------
        with tc.tile_pool(name="phC_q", bufs=2 * CT) as pcq, \
             tc.tile_pool(name="phC_p", bufs=1) as pcp, \
             tc.tile_pool(name="phC_pt", bufs=NJT // 4) as pcpt, \
             tc.tile_pool(name="phC_sm", bufs=8) as pcsm, \
             tc.tile_pool(name="phC_o", bufs=2) as pco, \
             tc.tile_pool(name="phC_r", bufs=8) as pcr, \
             tc.tile_pool(name="ps_s", bufs=2, space="PSUM") as pss, \
             tc.tile_pool(name="ps_t", bufs=1, space="PSUM") as pstp, \
             tc.tile_pool(name="ps_l", bufs=1, space="PSUM") as psl, \
             tc.tile_pool(name="ps_o", bufs=1, space="PSUM") as pso, \
             tc.tile_pool(name="ps_ot", bufs=1, space="PSUM") as psot, \
             tc.tile_pool(name="ps_z", bufs=2, space="PSUM") as psz:
            for it in range(NITILE):
                isl = slice(128 * it, 128 * (it + 1))
                qi = []
                for ci in range(CT):
                    qt = pcq.tile([128, 128], F32R, tag="qi")
                    nc.sync.dma_start(out=qt, in_=q_dram[128 * ci:128 * (ci + 1), isl])
                    qi.append(qt)
                # scores + exp
                p = pcp.tile([128, T], F32R, tag="p")
                for jc in range(NCHUNK):
                    ps = pss.tile([128, 512], F32, tag="ps_s")
                    for ci in range(CT):
                        nc.tensor.matmul(
                            ps, qi[ci], K_sb[ci][:, 512 * jc:512 * (jc + 1)],
                            start=(ci == 0), stop=(ci == CT - 1))
                    nc.scalar.activation(
                        out=p[:, 512 * jc:512 * (jc + 1)], in_=ps, func=Exp, scale=1.0)
                # transpose p blockwise (4 blocks per psum bank)
                pt4 = []
                for jg in range(NJT // 4):
                    pst_t = pstp.tile([128, 512], F32R, tag="ps_t")
                    for k in range(4):
                        jt = 4 * jg + k
                        nc.tensor.transpose(
                            pst_t[:, 128 * k:128 * (k + 1)],
                            p[:, 128 * jt:128 * (jt + 1)], ident)
                    ptt = pcpt.tile([128, 512], F32R, tag="pt4")
                    nc.vector.tensor_copy(ptt, pst_t.bitcast(F32))
                    pt4.append(ptt)
                # attn @ V  and row sums l
                ps_o = pso.tile([128, 512], F32, tag="ps_o")
                ps_l = psl.tile([128, 2], F32, tag="ps_l")
                for jt in range(NJT):
                    lhs = pt4[jt // 4][:, 128 * (jt % 4):128 * (jt % 4 + 1)]
                    nc.tensor.matmul(ps_o, lhs, V_sb[jt],
                                     start=(jt == 0), stop=(jt == NJT - 1))
                    nc.tensor.matmul(ps_l, lhs, ones_c,
                                     start=(jt == 0), stop=(jt == NJT - 1))
                r_sb = pcsm.tile([128, 1], F32, tag="r")
                nc.vector.reciprocal(r_sb, ps_l[:, 0:1])
                o_sb = pco.tile([128, 512], F32R, tag="o")
                nc.vector.tensor_scalar(out=o_sb, in0=ps_o, scalar1=r_sb,
                                        scalar2=None, op0=Alu.mult)
                # transpose attn output -> [c, i]
                ps_ot = psot.tile([128, 512], F32R, tag="ps_ot")
                for k in range(CT):
                    nc.tensor.transpose(
                        ps_ot[:, 128 * k:128 * (k + 1)],
                        o_sb[:, 128 * k:128 * (k + 1)], ident)
                ot_sb = pco.tile([128, 512], F32R, tag="ot")
                nc.vector.tensor_copy(ot_sb, ps_ot.bitcast(F32))
                # proj + bias + residual
                for co in range(CT):
                    ps_z = psz.tile([128, 128], F32, tag="ps_z")
                    for ci in range(CT):
                        nc.tensor.matmul(
                            ps_z, wp_sb[ci][:, 128 * co:128 * (co + 1)],
                            ot_sb[:, 128 * ci:128 * (ci + 1)],
                            start=(ci == 0), stop=False)
                    nc.tensor.matmul(ps_z, pb[0:1, 128 * co:128 * (co + 1)],
                                     ones_r[0:1, 0:128], start=False, stop=True)
                    xr = pcr.tile([128, 128], F32, tag="xr")
                    nc.sync.dma_start(out=xr, in_=x_l[128 * co:128 * (co + 1), isl])
                    zo = pcr.tile([128, 128], F32, tag="zo")
                    nc.vector.tensor_add(zo, ps_z, xr)
                    nc.sync.dma_start(out=out_l[128 * co:128 * (co + 1), isl], in_=zo)
    return nc


def _build():
    if "nc" in _CACHE:
        return _CACHE["nc"]
    nc = bacc.Bacc()
    _emit(nc)
    nc.compile()
    _CACHE["nc"] = nc
    return nc


def make_in_maps(x, gn_gamma, gn_beta, q_w, q_b, k_w, k_b, v_w, v_b, proj_w, proj_b):
    x = np.asarray(x, dtype=np.float32)
    scale = float(C) ** -0.5
    shared = {
        "wqT": np.ascontiguousarray(np.asarray(q_w, np.float32).T * scale),
        "wkT": np.ascontiguousarray(np.asarray(k_w, np.float32).T),
        "wvT": np.ascontiguousarray(np.asarray(v_w, np.float32).T),
        "wpT": np.ascontiguousarray(np.asarray(proj_w, np.float32).T),
        "qb_cols": (np.asarray(q_b, np.float32) * scale).reshape(CT, 128, 1),
        "kb_cols": np.asarray(k_b, np.float32).reshape(CT, 128, 1),
        "vb_row": np.asarray(v_b, np.float32).reshape(1, C),
        "pb_row": np.asarray(proj_b, np.float32).reshape(1, C),
        "gam_cols": np.asarray(gn_gamma, np.float32).reshape(CT, 128, 1),
        "bet_cols": np.asarray(gn_beta, np.float32).reshape(CT, 128, 1),
        "mask16": np.repeat(np.eye(NG_LOCAL, dtype=np.float32) / 16.0, 16, axis=0),
        "maskbc": np.repeat(np.eye(NG_LOCAL, dtype=np.float32), 16, axis=1),
        "ident": np.eye(128, dtype=np.float32),
        "ones_col": np.ones((128, 2), np.float32),
        "ones_row": np.ones((1, 512), np.float32),
    }
    in_maps = []
    for core in range(8):
        b, half = core // 2, core % 2
        x2d = x[b].reshape(C, T)
        x_loc = np.ascontiguousarray(
            np.concatenate([x2d[:, half * HALF:], x2d[:, :half * HALF]], axis=1))
        in_maps.append({"x_local": x_loc, **shared})
    return in_maps


def assemble_output(results):
    out = np.empty((B, C, Hh, Ww), np.float32)
    o2 = out.reshape(B, C, T)
    for core in range(8):
        b, half = core // 2, core % 2
        o2[b][:, half * HALF:(half + 1) * HALF] = results[core]["out_local"]
    return out


def get_runner():
    """Build (once) and return a callable in_maps -> per-core results list."""
    if "runner" in _CACHE:
        return _CACHE["runner"]
    nc = _build()
    from concourse import bass2jax

    def run(in_maps):
        return bass2jax.run_bass_via_pjrt(nc, in_maps, n_cores=8)

    _CACHE["runner"] = run
    return run


def kernel(**inputs) -> np.ndarray:
    in_maps = make_in_maps(**inputs)
    results = get_runner()(in_maps)
    return assemble_output(results)


# revision 5
# speedup vs baseline: 157.2328x; 157.2328x over previous
"""AttentionBlock kernel for 8 Trainium2 NeuronCores.

Reference computation (per batch b):
    h = GroupNorm32(x);  q,k,v = 1x1 conv(h);  single-head attention over
    hw=4096 tokens with C=512 channels;  out = x + proj(attn_out).

Sharding: 8 cores = 4 batches x 2 query-halves. Each core gets its batch's
x pre-rotated so its 2048 query tokens sit at columns [0, 2048) (attention
and groupnorm are permutation-invariant over tokens, so rotating keys/values
together is exact). Each core computes groupnorm + K/V for all 4096 tokens
and Q/attention/proj for its 2048 queries.

All matmuls run as float32r (full-rate fp32 mode, ~1e-4 rounding).
"""
import sys

for _p in ("/opt/trn_rl_repo", "/root/.axon_site/_ro/trn_rl_repo"):
    if _p not in sys.path:
        sys.path.append(_p)

import numpy as np

import concourse.bass as bass  # noqa: F401  (registers types)
import concourse.tile as tile
from concourse import bacc, mybir
from contextlib import ExitStack

F32 = mybir.dt.float32
F32R = mybir.dt.float32r

B, C, Hh, Ww = 4, 512, 64, 64
T = Hh * Ww            # 4096 tokens
HALF = T // 2          # 2048 queries per core
CT = C // 128          # 4 channel tiles
NCHUNK = T // 512      # 8 column chunks
NQCHUNK = HALF // 512  # 4 query chunks
NITILE = HALF // 128   # 16 query i-tiles
NJT = T // 128         # 32 key j-tiles
NG_LOCAL = 8           # groups per 128-channel tile (group size 16)
EPS = 1e-5

_CACHE = {}


def _emit(nc):
    x_l = nc.declare_dram_parameter("x_local", [C, T], F32, isOutput=False)
    wqT = nc.declare_dram_parameter("wqT", [C, C], F32R, isOutput=False)
    wkT = nc.declare_dram_parameter("wkT", [C, C], F32R, isOutput=False)
    wvT = nc.declare_dram_parameter("wvT", [C, C], F32R, isOutput=False)
    wpT = nc.declare_dram_parameter("wpT", [C, C], F32R, isOutput=False)
    qb_d = nc.declare_dram_parameter("qb_cols", [CT, 128, 1], F32, isOutput=False)
    kb_d = nc.declare_dram_parameter("kb_cols", [CT, 128, 1], F32, isOutput=False)
    vb_d = nc.declare_dram_parameter("vb_row", [1, C], F32R, isOutput=False)
    pb_d = nc.declare_dram_parameter("pb_row", [1, C], F32R, isOutput=False)
    gam_d = nc.declare_dram_parameter("gam_cols", [CT, 128, 1], F32, isOutput=False)
    bet_d = nc.declare_dram_parameter("bet_cols", [CT, 128, 1], F32, isOutput=False)
    m16_d = nc.declare_dram_parameter("mask16", [128, NG_LOCAL], F32, isOutput=False)
    mbc_d = nc.declare_dram_parameter("maskbc", [NG_LOCAL, 128], F32, isOutput=False)
    id_d = nc.declare_dram_parameter("ident", [128, 128], F32R, isOutput=False)
    onc_d = nc.declare_dram_parameter("ones_col", [128, 2], F32R, isOutput=False)
    onr_d = nc.declare_dram_parameter("ones_row", [1, 512], F32R, isOutput=False)
    out_l = nc.declare_dram_parameter("out_local", [C, HALF], F32, isOutput=True)

    q_dram = nc.dram_tensor("q_scratch", [C, HALF], F32R)

    Exp = mybir.ActivationFunctionType.Exp
    Sqrt = mybir.ActivationFunctionType.Sqrt
    Alu = mybir.AluOpType

    with tile.TileContext(nc) as tc, ExitStack() as ctx:
        consts = ctx.enter_context(tc.tile_pool(name="consts", bufs=1))
        wp_pool = ctx.enter_context(tc.tile_pool(name="wp", bufs=CT))
        k_pool = ctx.enter_context(tc.tile_pool(name="K", bufs=CT))
        v_pool = ctx.enter_context(tc.tile_pool(name="V", bufs=NJT))

        # ---- constants -------------------------------------------------
        m16 = consts.tile([128, NG_LOCAL], F32, tag="m16")
        nc.sync.dma_start(out=m16, in_=m16_d[:, :])
        mbc = consts.tile([NG_LOCAL, 128], F32, tag="mbc")
        nc.sync.dma_start(out=mbc, in_=mbc_d[:, :])
        ident = consts.tile([128, 128], F32R, tag="ident")
        nc.sync.dma_start(out=ident, in_=id_d[:, :])
        ones_c = consts.tile([128, 2], F32R, tag="ones_c")
        nc.sync.dma_start(out=ones_c, in_=onc_d[:, :])
        ones_r = consts.tile([1, 512], F32R, tag="ones_r")
        nc.sync.dma_start(out=ones_r, in_=onr_d[:, :])
        vb = consts.tile([1, C], F32R, tag="vb")
        nc.sync.dma_start(out=vb, in_=vb_d[:, :])
        pb = consts.tile([1, C], F32R, tag="pb")
        nc.sync.dma_start(out=pb, in_=pb_d[:, :])
        gam = consts.tile([128, CT], F32, tag="gam")
        bet = consts.tile([128, CT], F32, tag="bet")
        qb = consts.tile([128, CT], F32, tag="qb")
        kb = consts.tile([128, CT], F32, tag="kb")
        for t in range(CT):
            nc.sync.dma_start(out=gam[:, t:t + 1], in_=gam_d[t])
            nc.sync.dma_start(out=bet[:, t:t + 1], in_=bet_d[t])
            nc.sync.dma_start(out=qb[:, t:t + 1], in_=qb_d[t])
            nc.sync.dma_start(out=kb[:, t:t + 1], in_=kb_d[t])
        eps8 = consts.tile([NG_LOCAL, 1], F32, tag="eps8")
        nc.vector.memset(eps8, EPS)
        # groupnorm per-channel affine (filled by phase A)
        Ac = consts.tile([128, CT], F32, tag="Ac")
        Bc = consts.tile([128, CT], F32, tag="Bc")

        # ---- phase A: groupnorm statistics -----------------------------
        with tc.tile_pool(name="phA", bufs=6) as pha, \
             tc.tile_pool(name="phA_st", bufs=CT) as pst, \
             tc.tile_pool(name="phA_sm", bufs=2) as psm, \
             tc.tile_pool(name="phA_ps", bufs=1, space="PSUM") as pps:
            stats = [pst.tile([128, NCHUNK, 6], F32, tag="st", name="st") for _ in range(CT)]
            for jc in range(NCHUNK):
                for ci in range(CT):
                    xt = pha.tile([128, 512], F32, tag="xa")
                    nc.sync.dma_start(
                        out=xt, in_=x_l[128 * ci:128 * (ci + 1), 512 * jc:512 * (jc + 1)])
                    nc.vector.bn_stats(out=stats[ci][:, jc, :], in_=xt)
            # per-channel mean / E[x^2] -> per-group stats via PE mask matmuls
            ps_gm = pps.tile([NG_LOCAL, CT], F32, tag="gm")
            ps_gq = pps.tile([NG_LOCAL, CT], F32, tag="gq")
            for ci in range(CT):
                mv = psm.tile([128, 2], F32, tag="mv")
                nc.vector.bn_aggr(out=mv, in_=stats[ci])
                msq = psm.tile([128, 1], F32, tag="msq")
                nc.vector.tensor_mul(msq, mv[:, 0:1], mv[:, 0:1])
                qp = psm.tile([128, 1], F32, tag="qp")
                nc.vector.tensor_add(qp, mv[:, 1:2], msq)
                nc.tensor.matmul(ps_gm[:, ci:ci + 1], m16, mv[:, 0:1],
                                 start=(ci == 0), stop=(ci == CT - 1))
                nc.tensor.matmul(ps_gq[:, ci:ci + 1], m16, qp,
                                 start=(ci == 0), stop=(ci == CT - 1))
            sgm = psm.tile([NG_LOCAL, CT], F32, tag="sgm")
            nc.vector.tensor_copy(sgm, ps_gm)
            gvar = psm.tile([NG_LOCAL, CT], F32, tag="gvar")
            nc.vector.tensor_mul(gvar, sgm, sgm)
            nc.vector.tensor_sub(gvar, ps_gq, gvar)
            grstd = psm.tile([NG_LOCAL, CT], F32, tag="grstd")
            nc.scalar.activation(out=grstd, in_=gvar, func=Sqrt, bias=eps8, scale=1.0)
            nc.vector.reciprocal(grstd, grstd)
            # broadcast group stats back to channels, fold gamma/beta
            for ci in range(CT):
                ps_bm = pps.tile([128, 1], F32, tag="bm")
                ps_br = pps.tile([128, 1], F32, tag="br")
                nc.tensor.matmul(ps_bm, mbc, sgm[:, ci:ci + 1], start=True, stop=True)
                nc.tensor.matmul(ps_br, mbc, grstd[:, ci:ci + 1], start=True, stop=True)
                nc.vector.tensor_mul(Ac[:, ci:ci + 1], ps_br, gam[:, ci:ci + 1])
                tmp = psm.tile([128, 1], F32, tag="tmp")
                nc.vector.tensor_mul(tmp, ps_bm, Ac[:, ci:ci + 1])
                nc.vector.tensor_sub(Bc[:, ci:ci + 1], bet[:, ci:ci + 1], tmp)

        # ---- phase B: h = affine(x); K, V^T, Q projections -------------
        K_sb = [k_pool.tile([128, T], F32R, tag="K", name="K") for _ in range(CT)]
        V_sb = [v_pool.tile([128, 512], F32R, tag="V", name="V") for _ in range(NJT)]
        wp_sb = [wp_pool.tile([128, C], F32R, tag="wpT", name="wpT") for _ in range(CT)]
        for ci in range(CT):
            nc.sync.dma_start(out=wp_sb[ci], in_=wpT[128 * ci:128 * (ci + 1), :])

        with tc.tile_pool(name="phB_w", bufs=3 * CT) as pbw, \
             tc.tile_pool(name="phB_x", bufs=6) as pbx, \
             tc.tile_pool(name="phB_h", bufs=6) as pbh, \
             tc.tile_pool(name="phB_q", bufs=2) as pbq, \
             tc.tile_pool(name="phB_ps", bufs=3, space="PSUM") as pbp:
            wq_sb = [pbw.tile([128, C], F32R, tag="wT", name="wT") for _ in range(CT)]
            wk_sb = [pbw.tile([128, C], F32R, tag="wT", name="wT") for _ in range(CT)]
            wv_sb = [pbw.tile([128, C], F32R, tag="wT", name="wT") for _ in range(CT)]
            for ci in range(CT):
                nc.sync.dma_start(out=wq_sb[ci], in_=wqT[128 * ci:128 * (ci + 1), :])
                nc.sync.dma_start(out=wk_sb[ci], in_=wkT[128 * ci:128 * (ci + 1), :])
                nc.sync.dma_start(out=wv_sb[ci], in_=wvT[128 * ci:128 * (ci + 1), :])

            for jc in range(NCHUNK):
                cs = slice(512 * jc, 512 * (jc + 1))
                hj = []
                for ci in range(CT):
                    xt = pbx.tile([128, 512], F32, tag="xb")
                    nc.sync.dma_start(out=xt, in_=x_l[128 * ci:128 * (ci + 1), cs])
                    ht = pbh.tile([128, 512], F32R, tag="hb")
                    nc.vector.tensor_scalar(
                        out=ht, in0=xt, scalar1=Ac[:, ci:ci + 1],
                        scalar2=Bc[:, ci:ci + 1], op0=Alu.mult, op1=Alu.add)
                    hj.append(ht)
                # K[:, chunk]
                for co in range(CT):
                    ps = pbp.tile([128, 512], F32, tag="psb")
                    for ci in range(CT):
                        nc.tensor.matmul(
                            ps, wk_sb[ci][:, 128 * co:128 * (co + 1)], hj[ci],
                            start=(ci == 0), stop=(ci == CT - 1))
                    nc.vector.tensor_scalar(
                        out=K_sb[co][:, cs], in0=ps, scalar1=kb[:, co:co + 1],
                        scalar2=None, op0=Alu.add)
                # V^T tiles (4 per chunk)
                for ti in range(4):
                    jt = 4 * jc + ti
                    ps = pbp.tile([128, 512], F32, tag="psb")
                    for ci in range(CT):
                        nc.tensor.matmul(
                            ps, hj[ci][:, 128 * ti:128 * (ti + 1)], wv_sb[ci],
                            start=(ci == 0), stop=False)
                    nc.tensor.matmul(ps, ones_r[0:1, 0:128], vb[0:1, :],
                                     start=False, stop=True)
                    nc.vector.tensor_copy(V_sb[jt], ps)
                # Q[:, chunk] (first half only) -> DRAM scratch
                if jc < NQCHUNK:
                    for co in range(CT):
                        ps = pbp.tile([128, 512], F32, tag="psb")
                        for ci in range(CT):
                            nc.tensor.matmul(
                                ps, wq_sb[ci][:, 128 * co:128 * (co + 1)], hj[ci],
                                start=(ci == 0), stop=(ci == CT - 1))
                        qt = pbq.tile([128, 512], F32R, tag="qs")
                        nc.vector.tensor_scalar(
                            out=qt, in0=ps, scalar1=qb[:, co:co + 1],
                            scalar2=None, op0=Alu.add)
                        nc.sync.dma_start(
                            out=q_dram[128 * co:128 * (co + 1), cs], in_=qt)

        # ---- phase C: attention + proj + residual ----------------------
        with tc.tile_pool(name="phC_q", bufs=2 * CT) as pcq, \
             tc.tile_pool(name="phC_p", bufs=1) as pcp, \
             tc.tile_pool(name="phC_pt", bufs=NJT // 4) as pcpt, \
             tc.tile_pool(name="phC_sm", bufs=8) as pcsm, \
             tc.tile_pool(name="phC_o", bufs=2) as pco, \
             tc.tile_pool(name="phC_r", bufs=8) as pcr, \
             tc.tile_pool(name="ps_s", bufs=2, space="PSUM") as pss, \
             tc.tile_pool(name="ps_t", bufs=1, space="PSUM") as pstp, \
             tc.tile_pool(name="ps_l", bufs=1, space="PSUM") as psl, \
             tc.tile_pool(name="ps_o", bufs=1, space="PSUM") as pso, \
             tc.tile_pool(name="ps_ot", bufs=1, space="PSUM") as psot, \
             tc.tile_pool(name="ps_z", bufs=2, space="PSUM") as psz:
            for it in range(NITILE):
                isl = slice(128 * it, 128 * (it + 1))
                qi = []
                for ci in range(CT):
                    qt = pcq.tile([128, 128], F32R, tag="qi")
                    nc.sync.dma_start(out=qt, in_=q_dram[128 * ci:128 * (ci + 1), isl])
                    qi.append(qt)
                # scores + exp
                p = pcp.tile([128, T], F32R, tag="p")
                for jc in range(NCHUNK):
                    ps = pss.tile([128, 512], F32, tag="ps_s")
                    for ci in range(CT):
                        nc.tensor.matmul(
                            ps, qi[ci], K_sb[ci][:, 512 * jc:512 * (jc + 1)],
                            start=(ci == 0), stop=(ci == CT - 1))
                    nc.scalar.activation(
                        out=p[:, 512 * jc:512 * (jc + 1)], in_=ps, func=Exp, scale=1.0)
                # transpose p blockwise (4 blocks per psum bank)
                pt4 = []
                for jg in range(NJT // 4):
                    pst_t = pstp.tile([128, 512], F32R, tag="ps_t")
                    for k in range(4):
                        jt = 4 * jg + k
                        nc.tensor.transpose(
                            pst_t[:, 128 * k:128 * (k + 1)],
                            p[:, 128 * jt:128 * (jt + 1)], ident)
                    ptt = pcpt.tile([128, 512], F32R, tag="pt4")
                    nc.vector.tensor_copy(ptt, pst_t.bitcast(F32))
                    pt4.append(ptt)
                # attn @ V  and row sums l
                ps_o = pso.tile([128, 512], F32, tag="ps_o")
                ps_l = psl.tile([128, 2], F32, tag="ps_l")
                for jt in range(NJT):
                    lhs = pt4[jt // 4][:, 128 * (jt % 4):128 * (jt % 4 + 1)]
                    nc.tensor.matmul(ps_o, lhs, V_sb[jt],
                                     start=(jt == 0), stop=(jt == NJT - 1))
                    nc.tensor.matmul(ps_l, lhs, ones_c,
                                     start=(jt == 0), stop=(jt == NJT - 1))
                r_sb = pcsm.tile([128, 1], F32, tag="r")
                nc.vector.reciprocal(r_sb, ps_l[:, 0:1])
                o_sb = pco.tile([128, 512], F32R, tag="o")
                nc.vector.tensor_scalar(out=o_sb, in0=ps_o, scalar1=r_sb,
                                        scalar2=None, op0=Alu.mult)
                # transpose attn output -> [c, i]
                ps_ot = psot.tile([128, 512], F32R, tag="ps_ot")
                for k in range(CT):
                    nc.tensor.transpose(
                        ps_ot[:, 128 * k:128 * (k + 1)],
                        o_sb[:, 128 * k:128 * (k + 1)], ident)
                ot_sb = pco.tile([128, 512], F32R, tag="ot")
                nc.vector.tensor_copy(ot_sb, ps_ot.bitcast(F32))
                # proj + bias + residual
                for co in range(CT):
                    ps_z = psz.tile([128, 128], F32, tag="ps_z")
                    for ci in range(CT):
                        nc.tensor.matmul(
                            ps_z, wp_sb[ci][:, 128 * co:128 * (co + 1)],
                            ot_sb[:, 128 * ci:128 * (ci + 1)],
                            start=(ci == 0), stop=False)
                    nc.tensor.matmul(ps_z, pb[0:1, 128 * co:128 * (co + 1)],
                                     ones_r[0:1, 0:128], start=False, stop=True)
                    xr = pcr.tile([128, 128], F32, tag="xr")
                    nc.sync.dma_start(out=xr, in_=x_l[128 * co:128 * (co + 1), isl])
                    zo = pcr.tile([128, 128], F32, tag="zo")
                    nc.vector.tensor_add(zo, ps_z, xr)
                    nc.sync.dma_start(out=out_l[128 * co:128 * (co + 1), isl], in_=zo)
    return nc


def _build():
    if "nc" in _CACHE:
        return _CACHE["nc"]
    nc = bacc.Bacc()
    _emit(nc)
    nc.compile()
    _CACHE["nc"] = nc
    return nc


def make_in_maps(x, gn_gamma, gn_beta, q_w, q_b, k_w, k_b, v_w, v_b, proj_w, proj_b):
    x = np.asarray(x, dtype=np.float32)
    scale = float(C) ** -0.5
    shared = {
        "wqT": np.ascontiguousarray(np.asarray(q_w, np.float32).T * scale),
        "wkT": np.ascontiguousarray(np.asarray(k_w, np.float32).T),
        "wvT": np.ascontiguousarray(np.asarray(v_w, np.float32).T),
        "wpT": np.ascontiguousarray(np.asarray(proj_w, np.float32).T),
        "qb_cols": (np.asarray(q_b, np.float32) * scale).reshape(CT, 128, 1),
        "kb_cols": np.asarray(k_b, np.float32).reshape(CT, 128, 1),
        "vb_row": np.asarray(v_b, np.float32).reshape(1, C),
        "pb_row": np.asarray(proj_b, np.float32).reshape(1, C),
        "gam_cols": np.asarray(gn_gamma, np.float32).reshape(CT, 128, 1),
        "bet_cols": np.asarray(gn_beta, np.float32).reshape(CT, 128, 1),
        "mask16": np.repeat(np.eye(NG_LOCAL, dtype=np.float32) / 16.0, 16, axis=0),
        "maskbc": np.repeat(np.eye(NG_LOCAL, dtype=np.float32), 16, axis=1),
        "ident": np.eye(128, dtype=np.float32),
        "ones_col": np.ones((128, 2), np.float32),
        "ones_row": np.ones((1, 512), np.float32),
    }
    in_maps = []
    for core in range(8):
        b, half = core // 2, core % 2
        x2d = x[b].reshape(C, T)
        x_loc = np.ascontiguousarray(
            np.concatenate([x2d[:, half * HALF:], x2d[:, :half * HALF]], axis=1))
        in_maps.append({"x_local": x_loc, **shared})
    return in_maps


def assemble_output(results):
    out = np.empty((B, C, Hh, Ww), np.float32)
    o2 = out.reshape(B, C, T)
    for core in range(8):
        b, half = core // 2, core % 2
        o2[b][:, half * HALF:(half + 1) * HALF] = results[core]["out_local"]
    return out


def get_runner():
    """Build (once) and return a callable in_maps -> per-core results list.

    Mirrors bass2jax.run_bass_via_pjrt but constructs the jitted shard_map
    callable once so repeated invocations skip retracing/recompiling.
    """
    if "runner" in _CACHE:
        return _CACHE["runner"]
    nc = _build()
    import jax
    import numpy as _np
    from jax.sharding import Mesh, PartitionSpec
    from jax.experimental.shard_map import shard_map
    from concourse import bass2jax, mybir as _mb
    bass2jax.install_neuronx_cc_hook()

    n_cores = 8
    partition_name = nc.partition_id_tensor.name if nc.partition_id_tensor else None
    in_names, out_names, out_avals, zero_outs = [], [], [], []
    for alloc in nc.m.functions[0].allocations:
        if not isinstance(alloc, _mb.MemoryLocationSet):
            continue
        name = alloc.memorylocations[0].name
        if alloc.kind == "ExternalInput":
            if name != partition_name:
                in_names.append(name)
        elif alloc.kind == "ExternalOutput":
            shape = tuple(alloc.tensor_shape)
            dtype = _mb.dt.np(alloc.dtype)
            out_names.append(name)
            out_avals.append(jax.core.ShapedArray(shape, dtype))
            zero_outs.append(_np.zeros(shape, dtype))
    n_params = len(in_names)
    n_outs = len(out_avals)
    all_in_names = list(in_names) + list(out_names)
    if partition_name is not None:
        all_in_names.append(partition_name)
    donate = tuple(range(n_params, n_params + n_outs))

    def _body(*args):
        operands = list(args)
        if partition_name is not None:
            operands.append(bass2jax.partition_id_tensor())
        outs = bass2jax._bass_exec_p.bind(
            *operands,
            out_avals=tuple(out_avals),
            in_names=tuple(all_in_names),
            out_names=tuple(out_names),
            lowering_input_output_aliases=(),
            sim_require_finite=True,
            sim_require_nnan=True,
            nc=nc,
        )
        return tuple(outs)

    devices = jax.devices()[:n_cores]
    mesh = Mesh(_np.asarray(devices), ("core",))
    in_specs = (PartitionSpec("core"),) * (n_params + n_outs)
    out_specs = (PartitionSpec("core"),) * n_outs
    sharded = jax.jit(
        shard_map(_body, mesh=mesh, in_specs=in_specs, out_specs=out_specs,
                  check_rep=False),
        donate_argnums=donate, keep_unused=True)

    def prep_inputs(in_maps):
        """Concatenate per-core inputs along axis 0 (host-side)."""
        return [
            _np.concatenate([_np.asarray(in_maps[c][nm]) for c in range(n_cores)], axis=0)
            for nm in in_names
        ]

    def make_zeros():
        return [_np.zeros((n_cores * z.shape[0], *z.shape[1:]), z.dtype)
                for z in zero_outs]

    def run_prepared(concat_in, concat_zeros):
        out_arrs = sharded(*concat_in, *concat_zeros)
        return out_arrs

    def run(in_maps):
        out_arrs = run_prepared(prep_inputs(in_maps), make_zeros())
        return [
            {nm: _np.asarray(out_arrs[i]).reshape(n_cores, *out_avals[i].shape)[c]
             for i, nm in enumerate(out_names)}
            for c in range(n_cores)
        ]

    run.prep_inputs = prep_inputs
    run.make_zeros = make_zeros
    run.run_prepared = run_prepared
    _CACHE["runner"] = run
    return run


def kernel(**inputs) -> np.ndarray:
    in_maps = make_in_maps(**inputs)
    results = get_runner()(in_maps)
    return assemble_output(results)


# revision 6
# speedup vs baseline: 461.5836x; 2.9357x over previous
"""AttentionBlock kernel for 8 Trainium2 NeuronCores.

Reference computation (per batch b):
    h = GroupNorm32(x);  q,k,v = 1x1 conv(h);  single-head attention over
    hw=4096 tokens with C=512 channels;  out = x + proj(attn_out).

Sharding: 8 cores = 4 batches x 2 query-halves. Each core gets its batch's
x pre-rotated so its 2048 query tokens sit at columns [0, 2048) (attention
and groupnorm are permutation-invariant over tokens, so rotating keys/values
together is exact). Each core computes groupnorm + K/V for all 4096 tokens
and Q/attention/proj for its 2048 queries.

All big matmuls run as float32r (full-rate fp32 PE mode, ~1e-4 rounding).
All per-core inputs are packed into a single flat f32 blob: the PJRT/axon
execute path pays a multi-ms fixed cost PER INPUT TENSOR, so one blob is
dramatically cheaper to stage than 17 separate parameters.
"""
import sys

for _p in ("/opt/trn_rl_repo", "/root/.axon_site/_ro/trn_rl_repo"):
    if _p not in sys.path:
        sys.path.append(_p)

import numpy as np

import concourse.bass as bass  # noqa: F401  (registers types)
import concourse.tile as tile
from concourse import bacc, mybir
from contextlib import ExitStack

F32 = mybir.dt.float32
F32R = mybir.dt.float32r

B, C, Hh, Ww = 4, 512, 64, 64
T = Hh * Ww            # 4096 tokens
HALF = T // 2          # 2048 queries per core
CT = C // 128          # 4 channel tiles
NCHUNK = T // 512      # 8 column chunks
NQCHUNK = HALF // 512  # 4 query chunks
NITILE = HALF // 128   # 16 query i-tiles
NJT = T // 128         # 32 key j-tiles
NG_LOCAL = 8           # groups per 128-channel tile (group size 16)
EPS = 1e-5

# blob layout: name -> (offset_in_floats, shape)
_LAYOUT = {}
_BLOB_SIZE = 0


def _lay(name, shape):
    global _BLOB_SIZE
    n = int(np.prod(shape))
    _LAYOUT[name] = (_BLOB_SIZE, tuple(shape))
    _BLOB_SIZE += n


_lay("x_local", (C, T))
_lay("wqT", (C, C))
_lay("wkT", (C, C))
_lay("wvT", (C, C))
_lay("wpT", (C, C))
_lay("qb_cols", (CT, 128, 1))
_lay("kb_cols", (CT, 128, 1))
_lay("vb_row", (1, C))
_lay("pb_row", (1, C))
_lay("gam_cols", (CT, 128, 1))
_lay("bet_cols", (CT, 128, 1))
_lay("mask16", (128, NG_LOCAL))
_lay("maskbc", (NG_LOCAL, 128))
_lay("ident", (128, 128))
_lay("ones_col", (128, 2))
_lay("ones_row", (1, 512))

_CACHE = {}


def _emit(nc):
    blob = nc.declare_dram_parameter("blob", [_BLOB_SIZE], F32, isOutput=False)
    out_l = nc.declare_dram_parameter("out_local", [C, HALF], F32, isOutput=True)

    def view(name, f32r=False):
        off, shape = _LAYOUT[name]
        ap = blob[off:off + int(np.prod(shape))]
        if len(shape) == 2:
            ap = ap.rearrange("(a b) -> a b", b=shape[1])
        elif len(shape) == 3:
            ap = ap.rearrange("(a b c) -> a b c", b=shape[1], c=shape[2])
        return ap.bitcast(F32R) if f32r else ap

    x_l = view("x_local")
    wqT, wkT = view("wqT", True), view("wkT", True)
    wvT, wpT = view("wvT", True), view("wpT", True)

    q_dram = nc.dram_tensor("q_scratch", [C, HALF], F32R)

    Exp = mybir.ActivationFunctionType.Exp
    Sqrt = mybir.ActivationFunctionType.Sqrt
    Alu = mybir.AluOpType

    with tile.TileContext(nc) as tc, ExitStack() as ctx:
        consts = ctx.enter_context(tc.tile_pool(name="consts", bufs=1))
        wp_pool = ctx.enter_context(tc.tile_pool(name="wp", bufs=CT))
        k_pool = ctx.enter_context(tc.tile_pool(name="K", bufs=CT))
        v_pool = ctx.enter_context(tc.tile_pool(name="V", bufs=NJT))

        # ---- constants -------------------------------------------------
        m16 = consts.tile([128, NG_LOCAL], F32, tag="m16")
        nc.sync.dma_start(out=m16, in_=view("mask16"))
        mbc = consts.tile([NG_LOCAL, 128], F32, tag="mbc")
        nc.sync.dma_start(out=mbc, in_=view("maskbc"))
        ident = consts.tile([128, 128], F32R, tag="ident")
        nc.sync.dma_start(out=ident, in_=view("ident", True))
        ones_c = consts.tile([128, 2], F32R, tag="ones_c")
        nc.sync.dma_start(out=ones_c, in_=view("ones_col", True))
        ones_r = consts.tile([1, 512], F32R, tag="ones_r")
        nc.sync.dma_start(out=ones_r, in_=view("ones_row", True))
        vb = consts.tile([1, C], F32R, tag="vb")
        nc.sync.dma_start(out=vb, in_=view("vb_row", True))
        pb = consts.tile([1, C], F32R, tag="pb")
        nc.sync.dma_start(out=pb, in_=view("pb_row", True))
        gam = consts.tile([128, CT], F32, tag="gam")
        bet = consts.tile([128, CT], F32, tag="bet")
        qb = consts.tile([128, CT], F32, tag="qb")
        kb = consts.tile([128, CT], F32, tag="kb")
        gam_v, bet_v = view("gam_cols"), view("bet_cols")
        qb_v, kb_v = view("qb_cols"), view("kb_cols")
        for t in range(CT):
            nc.sync.dma_start(out=gam[:, t:t + 1], in_=gam_v[t])
            nc.sync.dma_start(out=bet[:, t:t + 1], in_=bet_v[t])
            nc.sync.dma_start(out=qb[:, t:t + 1], in_=qb_v[t])
            nc.sync.dma_start(out=kb[:, t:t + 1], in_=kb_v[t])
        eps8 = consts.tile([NG_LOCAL, 1], F32, tag="eps8")
        nc.vector.memset(eps8, EPS)
        # groupnorm per-channel affine (filled by phase A)
        Ac = consts.tile([128, CT], F32, tag="Ac")
        Bc = consts.tile([128, CT], F32, tag="Bc")

        # ---- phase A: groupnorm statistics -----------------------------
        with tc.tile_pool(name="phA", bufs=6) as pha, \
             tc.tile_pool(name="phA_st", bufs=CT) as pst, \
             tc.tile_pool(name="phA_sm", bufs=2) as psm, \
             tc.tile_pool(name="phA_ps", bufs=1, space="PSUM") as pps:
            stats = [pst.tile([128, NCHUNK, 6], F32, tag="st", name="st")
                     for _ in range(CT)]
            for jc in range(NCHUNK):
                for ci in range(CT):
                    xt = pha.tile([128, 512], F32, tag="xa")
                    nc.sync.dma_start(
                        out=xt,
                        in_=x_l[128 * ci:128 * (ci + 1), 512 * jc:512 * (jc + 1)])
                    nc.vector.bn_stats(out=stats[ci][:, jc, :], in_=xt)
            # per-channel mean / E[x^2] -> per-group stats via PE mask matmuls
            ps_gm = pps.tile([NG_LOCAL, CT], F32, tag="gm")
            ps_gq = pps.tile([NG_LOCAL, CT], F32, tag="gq")
            for ci in range(CT):
                mv = psm.tile([128, 2], F32, tag="mv")
                nc.vector.bn_aggr(out=mv, in_=stats[ci])
                msq = psm.tile([128, 1], F32, tag="msq")
                nc.vector.tensor_mul(msq, mv[:, 0:1], mv[:, 0:1])
                qp = psm.tile([128, 1], F32, tag="qp")
                nc.vector.tensor_add(qp, mv[:, 1:2], msq)
                nc.tensor.matmul(ps_gm[:, ci:ci + 1], m16, mv[:, 0:1],
                                 start=(ci == 0), stop=(ci == CT - 1))
                nc.tensor.matmul(ps_gq[:, ci:ci + 1], m16, qp,
                                 start=(ci == 0), stop=(ci == CT - 1))
            sgm = psm.tile([NG_LOCAL, CT], F32, tag="sgm")
            nc.vector.tensor_copy(sgm, ps_gm)
            gvar = psm.tile([NG_LOCAL, CT], F32, tag="gvar")
            nc.vector.tensor_mul(gvar, sgm, sgm)
            nc.vector.tensor_sub(gvar, ps_gq, gvar)
            grstd = psm.tile([NG_LOCAL, CT], F32, tag="grstd")
            nc.scalar.activation(out=grstd, in_=gvar, func=Sqrt, bias=eps8, scale=1.0)
            nc.vector.reciprocal(grstd, grstd)
            # broadcast group stats back to channels, fold gamma/beta
            for ci in range(CT):
                ps_bm = pps.tile([128, 1], F32, tag="bm")
                ps_br = pps.tile([128, 1], F32, tag="br")
                nc.tensor.matmul(ps_bm, mbc, sgm[:, ci:ci + 1], start=True, stop=True)
                nc.tensor.matmul(ps_br, mbc, grstd[:, ci:ci + 1], start=True, stop=True)
                nc.vector.tensor_mul(Ac[:, ci:ci + 1], ps_br, gam[:, ci:ci + 1])
                tmp = psm.tile([128, 1], F32, tag="tmp")
                nc.vector.tensor_mul(tmp, ps_bm, Ac[:, ci:ci + 1])
                nc.vector.tensor_sub(Bc[:, ci:ci + 1], bet[:, ci:ci + 1], tmp)

        # ---- phase B: h = affine(x); K, V^T, Q projections -------------
        K_sb = [k_pool.tile([128, T], F32R, tag="K", name="K") for _ in range(CT)]
        V_sb = [v_pool.tile([128, 512], F32R, tag="V", name="V") for _ in range(NJT)]
        wp_sb = [wp_pool.tile([128, C], F32R, tag="wpT", name="wpT")
                 for _ in range(CT)]
        for ci in range(CT):
            nc.sync.dma_start(out=wp_sb[ci], in_=wpT[128 * ci:128 * (ci + 1), :])

        with tc.tile_pool(name="phB_w", bufs=3 * CT) as pbw, \
             tc.tile_pool(name="phB_x", bufs=6) as pbx, \
             tc.tile_pool(name="phB_h", bufs=6) as pbh, \
             tc.tile_pool(name="phB_q", bufs=2) as pbq, \
             tc.tile_pool(name="phB_ps", bufs=3, space="PSUM") as pbp:
            wq_sb = [pbw.tile([128, C], F32R, tag="wT", name="wT") for _ in range(CT)]
            wk_sb = [pbw.tile([128, C], F32R, tag="wT", name="wT") for _ in range(CT)]
            wv_sb = [pbw.tile([128, C], F32R, tag="wT", name="wT") for _ in range(CT)]
            for ci in range(CT):
                nc.sync.dma_start(out=wq_sb[ci], in_=wqT[128 * ci:128 * (ci + 1), :])
                nc.sync.dma_start(out=wk_sb[ci], in_=wkT[128 * ci:128 * (ci + 1), :])
                nc.sync.dma_start(out=wv_sb[ci], in_=wvT[128 * ci:128 * (ci + 1), :])

            for jc in range(NCHUNK):
                cs = slice(512 * jc, 512 * (jc + 1))
                hj = []
                for ci in range(CT):
                    xt = pbx.tile([128, 512], F32, tag="xb")
                    nc.sync.dma_start(out=xt, in_=x_l[128 * ci:128 * (ci + 1), cs])
                    ht = pbh.tile([128, 512], F32R, tag="hb")
                    nc.vector.tensor_scalar(
                        out=ht, in0=xt, scalar1=Ac[:, ci:ci + 1],
                        scalar2=Bc[:, ci:ci + 1], op0=Alu.mult, op1=Alu.add)
                    hj.append(ht)
                # K[:, chunk]
                for co in range(CT):
                    ps = pbp.tile([128, 512], F32, tag="psb")
                    for ci in range(CT):
                        nc.tensor.matmul(
                            ps, wk_sb[ci][:, 128 * co:128 * (co + 1)], hj[ci],
                            start=(ci == 0), stop=(ci == CT - 1))
                    nc.vector.tensor_scalar(
                        out=K_sb[co][:, cs], in0=ps, scalar1=kb[:, co:co + 1],
                        scalar2=None, op0=Alu.add)
                # V^T tiles (4 per chunk)
                for ti in range(4):
                    jt = 4 * jc + ti
                    ps = pbp.tile([128, 512], F32, tag="psb")
                    for ci in range(CT):
                        nc.tensor.matmul(
                            ps, hj[ci][:, 128 * ti:128 * (ti + 1)], wv_sb[ci],
                            start=(ci == 0), stop=False)
                    nc.tensor.matmul(ps, ones_r[0:1, 0:128], vb[0:1, :],
                                     start=False, stop=True)
                    nc.vector.tensor_copy(V_sb[jt], ps)
                # Q[:, chunk] (first half only) -> DRAM scratch
                if jc < NQCHUNK:
                    for co in range(CT):
                        ps = pbp.tile([128, 512], F32, tag="psb")
                        for ci in range(CT):
                            nc.tensor.matmul(
                                ps, wq_sb[ci][:, 128 * co:128 * (co + 1)], hj[ci],
                                start=(ci == 0), stop=(ci == CT - 1))
                        qt = pbq.tile([128, 512], F32R, tag="qs")
                        nc.vector.tensor_scalar(
                            out=qt, in0=ps, scalar1=qb[:, co:co + 1],
                            scalar2=None, op0=Alu.add)
                        nc.sync.dma_start(
                            out=q_dram[128 * co:128 * (co + 1), cs], in_=qt)

        # ---- phase C: attention + proj + residual ----------------------
        with tc.tile_pool(name="phC_q", bufs=2 * CT) as pcq, \
             tc.tile_pool(name="phC_p", bufs=1) as pcp, \
             tc.tile_pool(name="phC_pt", bufs=NJT // 4) as pcpt, \
             tc.tile_pool(name="phC_sm", bufs=8) as pcsm, \
             tc.tile_pool(name="phC_o", bufs=2) as pco, \
             tc.tile_pool(name="phC_r", bufs=8) as pcr, \
             tc.tile_pool(name="ps_s", bufs=2, space="PSUM") as pss, \
             tc.tile_pool(name="ps_t", bufs=1, space="PSUM") as pstp, \
             tc.tile_pool(name="ps_l", bufs=1, space="PSUM") as psl, \
             tc.tile_pool(name="ps_o", bufs=1, space="PSUM") as pso, \
             tc.tile_pool(name="ps_ot", bufs=1, space="PSUM") as psot, \
             tc.tile_pool(name="ps_z", bufs=2, space="PSUM") as psz:
            for it in range(NITILE):
                isl = slice(128 * it, 128 * (it + 1))
                qi = []
                for ci in range(CT):
                    qt = pcq.tile([128, 128], F32R, tag="qi")
                    nc.sync.dma_start(out=qt, in_=q_dram[128 * ci:128 * (ci + 1), isl])
                    qi.append(qt)
                # scores + exp
                p = pcp.tile([128, T], F32R, tag="p")
                for jc in range(NCHUNK):
                    ps = pss.tile([128, 512], F32, tag="ps_s")
                    for ci in range(CT):
                        nc.tensor.matmul(
                            ps, qi[ci], K_sb[ci][:, 512 * jc:512 * (jc + 1)],
                            start=(ci == 0), stop=(ci == CT - 1))
                    nc.scalar.activation(
                        out=p[:, 512 * jc:512 * (jc + 1)], in_=ps, func=Exp, scale=1.0)
                # transpose p blockwise (4 blocks per psum bank)
                pt4 = []
                for jg in range(NJT // 4):
                    pst_t = pstp.tile([128, 512], F32R, tag="ps_t")
                    for k in range(4):
                        jt = 4 * jg + k
                        nc.tensor.transpose(
                            pst_t[:, 128 * k:128 * (k + 1)],
                            p[:, 128 * jt:128 * (jt + 1)], ident)
                    ptt = pcpt.tile([128, 512], F32R, tag="pt4", name="pt4")
                    nc.vector.tensor_copy(ptt, pst_t.bitcast(F32))
                    pt4.append(ptt)
                # attn @ V  and row sums l
                ps_o = pso.tile([128, 512], F32, tag="ps_o")
                ps_l = psl.tile([128, 2], F32, tag="ps_l")
                for jt in range(NJT):
                    lhs = pt4[jt // 4][:, 128 * (jt % 4):128 * (jt % 4 + 1)]
                    nc.tensor.matmul(ps_o, lhs, V_sb[jt],
                                     start=(jt == 0), stop=(jt == NJT - 1))
                    nc.tensor.matmul(ps_l, lhs, ones_c,
                                     start=(jt == 0), stop=(jt == NJT - 1))
                r_sb = pcsm.tile([128, 1], F32, tag="r")
                nc.vector.reciprocal(r_sb, ps_l[:, 0:1])
                o_sb = pco.tile([128, 512], F32R, tag="o")
                nc.vector.tensor_scalar(out=o_sb, in0=ps_o, scalar1=r_sb,
                                        scalar2=None, op0=Alu.mult)
                # transpose attn output -> [c, i]
                ps_ot = psot.tile([128, 512], F32R, tag="ps_ot")
                for k in range(CT):
                    nc.tensor.transpose(
                        ps_ot[:, 128 * k:128 * (k + 1)],
                        o_sb[:, 128 * k:128 * (k + 1)], ident)
                ot_sb = pco.tile([128, 512], F32R, tag="ot")
                nc.vector.tensor_copy(ot_sb, ps_ot.bitcast(F32))
                # proj + bias + residual
                for co in range(CT):
                    ps_z = psz.tile([128, 128], F32, tag="ps_z")
                    for ci in range(CT):
                        nc.tensor.matmul(
                            ps_z, wp_sb[ci][:, 128 * co:128 * (co + 1)],
                            ot_sb[:, 128 * ci:128 * (ci + 1)],
                            start=(ci == 0), stop=False)
                    nc.tensor.matmul(ps_z, pb[0:1, 128 * co:128 * (co + 1)],
                                     ones_r[0:1, 0:128], start=False, stop=True)
                    xr = pcr.tile([128, 128], F32, tag="xr")
                    nc.sync.dma_start(out=xr, in_=x_l[128 * co:128 * (co + 1), isl])
                    zo = pcr.tile([128, 128], F32, tag="zo")
                    nc.vector.tensor_add(zo, ps_z, xr)
                    nc.sync.dma_start(out=out_l[128 * co:128 * (co + 1), isl], in_=zo)
    return nc


def _build():
    if "nc" in _CACHE:
        return _CACHE["nc"]
    nc = bacc.Bacc(enable_partition_id=False)
    _emit(nc)
    nc.compile()
    _CACHE["nc"] = nc
    return nc


def _pack_blob(**arrays):
    blob = np.zeros(_BLOB_SIZE, np.float32)
    for name, arr in arrays.items():
        off, shape = _LAYOUT[name]
        a = np.asarray(arr, np.float32).reshape(shape)
        blob[off:off + a.size] = a.ravel()
    return blob


def make_in_maps(x, gn_gamma, gn_beta, q_w, q_b, k_w, k_b, v_w, v_b, proj_w, proj_b):
    x = np.asarray(x, dtype=np.float32)
    scale = float(C) ** -0.5
    shared = dict(
        wqT=np.ascontiguousarray(np.asarray(q_w, np.float32).T * scale),
        wkT=np.ascontiguousarray(np.asarray(k_w, np.float32).T),
        wvT=np.ascontiguousarray(np.asarray(v_w, np.float32).T),
        wpT=np.ascontiguousarray(np.asarray(proj_w, np.float32).T),
        qb_cols=(np.asarray(q_b, np.float32) * scale).reshape(CT, 128, 1),
        kb_cols=np.asarray(k_b, np.float32).reshape(CT, 128, 1),
        vb_row=np.asarray(v_b, np.float32).reshape(1, C),
        pb_row=np.asarray(proj_b, np.float32).reshape(1, C),
        gam_cols=np.asarray(gn_gamma, np.float32).reshape(CT, 128, 1),
        bet_cols=np.asarray(gn_beta, np.float32).reshape(CT, 128, 1),
        mask16=np.repeat(np.eye(NG_LOCAL, dtype=np.float32) / 16.0, 16, axis=0),
        maskbc=np.repeat(np.eye(NG_LOCAL, dtype=np.float32), 16, axis=1),
        ident=np.eye(128, dtype=np.float32),
        ones_col=np.ones((128, 2), np.float32),
        ones_row=np.ones((1, 512), np.float32),
    )
    in_maps = []
    for core in range(8):
        b, half = core // 2, core % 2
        x2d = x[b].reshape(C, T)
        x_loc = np.concatenate([x2d[:, half * HALF:], x2d[:, :half * HALF]], axis=1)
        in_maps.append({"blob": _pack_blob(x_local=x_loc, **shared)})
    return in_maps


def assemble_output(results):
    out = np.empty((B, C, Hh, Ww), np.float32)
    o2 = out.reshape(B, C, T)
    for core in range(8):
        b, half = core // 2, core % 2
        o2[b][:, half * HALF:(half + 1) * HALF] = results[core]["out_local"]
    return out


def get_runner():
    """Build (once) and return a callable in_maps -> per-core results list.

    Mirrors bass2jax.run_bass_via_pjrt but constructs the jitted shard_map
    callable once so repeated invocations skip retracing/recompiling.
    """
    if "runner" in _CACHE:
        return _CACHE["runner"]
    nc = _build()
    import jax
    import numpy as _np
    from jax.sharding import Mesh, PartitionSpec
    from jax.experimental.shard_map import shard_map
    from concourse import bass2jax, mybir as _mb
    bass2jax.install_neuronx_cc_hook()

    n_cores = 8
    partition_name = nc.partition_id_tensor.name if nc.partition_id_tensor else None
    in_names, out_names, out_avals, zero_outs = [], [], [], []
    for alloc in nc.m.functions[0].allocations:
        if not isinstance(alloc, _mb.MemoryLocationSet):
            continue
        name = alloc.memorylocations[0].name
        if alloc.kind == "ExternalInput":
            if name != partition_name:
                in_names.append(name)
        elif alloc.kind == "ExternalOutput":
            shape = tuple(alloc.tensor_shape)
            dtype = _mb.dt.np(alloc.dtype)
            out_names.append(name)
            out_avals.append(jax.core.ShapedArray(shape, dtype))
            zero_outs.append(_np.zeros(shape, dtype))
    n_params = len(in_names)
    n_outs = len(out_avals)
    all_in_names = list(in_names) + list(out_names)
    if partition_name is not None:
        all_in_names.append(partition_name)
    donate = tuple(range(n_params, n_params + n_outs))

    def _body(*args):
        operands = list(args)
        if partition_name is not None:
            operands.append(bass2jax.partition_id_tensor())
        outs = bass2jax._bass_exec_p.bind(
            *operands,
            out_avals=tuple(out_avals),
            in_names=tuple(all_in_names),
            out_names=tuple(out_names),
            lowering_input_output_aliases=(),
            sim_require_finite=True,
            sim_require_nnan=True,
            nc=nc,
        )
        return tuple(outs)

    devices = jax.devices()[:n_cores]
    mesh = Mesh(_np.asarray(devices), ("core",))
    in_specs = (PartitionSpec("core"),) * (n_params + n_outs)
    out_specs = (PartitionSpec("core"),) * n_outs
    sharded = jax.jit(
        shard_map(_body, mesh=mesh, in_specs=in_specs, out_specs=out_specs,
                  check_rep=False),
        donate_argnums=donate, keep_unused=True)

    def prep_inputs(in_maps):
        """Concatenate per-core inputs along axis 0 (host-side)."""
        return [
            _np.concatenate([_np.asarray(in_maps[c][nm]) for c in range(n_cores)],
                            axis=0)
            for nm in in_names
        ]

    def make_zeros():
        return [_np.zeros((n_cores * z.shape[0], *z.shape[1:]), z.dtype)
                for z in zero_outs]

    def run_prepared(concat_in, concat_zeros):
        return sharded(*concat_in, *concat_zeros)

    def run(in_maps):
        out_arrs = run_prepared(prep_inputs(in_maps), make_zeros())
        return [
            {nm: _np.asarray(out_arrs[i]).reshape(n_cores, *out_avals[i].shape)[c]
             for i, nm in enumerate(out_names)}
            for c in range(n_cores)
        ]

    run.prep_inputs = prep_inputs
    run.make_zeros = make_zeros
    run.run_prepared = run_prepared
    _CACHE["runner"] = run
    return run


def kernel(**inputs) -> np.ndarray:
    in_maps = make_in_maps(**inputs)
    results = get_runner()(in_maps)
    return assemble_output(results)
